# revision 4
# baseline (speedup 1.0000x reference)
"""Trainium2 Bass kernel for nn_HadamardExpansionV2 (topk_masking) — v2.

Sharding: data-parallel over batch B=16 across 8 cores (2 samples/core);
weights replicated. CrossHadaNorm batch stats via AllGather of per-channel
sum/sumsq (Gram-matrix trick) + local reduce.

Key structure vs v1 baseline:
  - conv + hadamard-gather matmuls in float32r (tf32, 1 cyc/row) with
    properly-rounded producers (DRAM tensors declared f32r; on-chip
    rounding copies where needed).
  - logits path fully exact fp32 (top-32 order is sensitive): host-folded
    Weff = (fc_w.T * s) @ eva_w.T / 1024, logits = xsum @ Weff + beff.
  - per-channel sum/sumsq of prod computed WITHOUT materializing prod:
    sum_hw(xsel_i * xsel_j) = Gram(xsel)[i, j]; sumsq via Gram(xsel^2).
    Gathered to the 496 pair channels via ghi/ghjT matmuls. This puts the
    collective early so it overlaps the hadamard A/B matmul + multiply.
  - conv epilogue (BN affine) on ACT (Identity, per-partition scale/bias)
    streaming to DRAM through a small staging pool.
  - final CrossHadaNorm affine split across DVE/ACT by m-parity, streamed
    to DRAM per chunk.
"""

import os
import sys

import numpy as np

for _p in ("/opt/trn_rl_repo", os.path.expanduser("~/.axon_site/_ro/trn_rl_repo")):
    if os.path.isdir(_p) and _p not in sys.path:
        sys.path.insert(0, _p)

import concourse.bass as bass
import concourse.mybir as mybir
import concourse.tile as tile
from concourse import bacc
from concourse.bass_utils import run_bass_kernel_spmd

C1 = 512
CS = 32
CSE = 496
HWD = 1024
B = 16
NCORES = 8
SPC = B // NCORES
P = 128
KC = C1 // P
MC = C1 // P
NF = 512
NNC = HWD // NF
EPS = 1e-5
NTOT = float(B * HWD)

HI, HJ = np.triu_indices(CS, k=1)

F32 = mybir.dt.float32
F32R = mybir.dt.float32r
U32 = mybir.dt.uint32
AF = mybir.ActivationFunctionType
ALU = mybir.AluOpType

EXP_M = [(0, 128), (128, 128), (256, 128), (384, 112)]
NHC = HWD // P  # 8 hw chunks of 128 for transposes


def build_program():
    nc = bacc.Bacc(
        "TRN2",
        target_bir_lowering=False,
        debug=False,
        num_devices=NCORES,
    )

    # ---------------- I/O ----------------
    xs = nc.dram_tensor("xs", [SPC, C1, HWD], F32, kind="ExternalInput")
    fc_wT = nc.dram_tensor("fc_wT", [C1, C1], F32R, kind="ExternalInput")  # [c, o]
    weff = nc.dram_tensor("weff", [C1, C1], F32, kind="ExternalInput")  # [c, o']
    beff = nc.dram_tensor("beff", [1, C1], F32, kind="ExternalInput")
    s_vec = nc.dram_tensor("s_vec", [P, MC], F32, kind="ExternalInput")
    b2_vec = nc.dram_tensor("b2_vec", [P, MC], F32, kind="ExternalInput")
    gpair_t = nc.dram_tensor("gpair_t", [P, MC], F32, kind="ExternalInput")
    bpair_t = nc.dram_tensor("bpair_t", [P, MC], F32, kind="ExternalInput")
    g_hi = nc.dram_tensor("g_hi", [CS, CSE], F32R, kind="ExternalInput")
    g_hj = nc.dram_tensor("g_hj", [CS, CSE], F32R, kind="ExternalInput")
    g_hjT = nc.dram_tensor("g_hjT", [P, MC, CS], F32, kind="ExternalInput")
    eye32 = nc.dram_tensor("eye32", [CS, CS], F32, kind="ExternalInput")
    eye1 = nc.dram_tensor("eye1", [1, 1], F32, kind="ExternalInput")

    outs = [
        nc.dram_tensor(f"out{s}", [C1 + CSE, HWD], F32, kind="ExternalOutput")
        for s in range(SPC)
    ]

    with tile.TileContext(nc) as tc:
        with (
            tc.tile_pool(name="const", bufs=1) as cpool,
            tc.tile_pool(name="xp", bufs=1) as xpool,
            tc.tile_pool(name="prodp", bufs=1) as prodpool,
            tc.tile_pool(name="ystage", bufs=3) as ypool,
            tc.tile_pool(name="ostage", bufs=3) as opool,
            tc.tile_pool(name="acopy", bufs=3) as apool,
            tc.tile_pool(name="small", bufs=2) as spool,
            tc.tile_pool(name="psc", bufs=3, space="PSUM") as psc,
            tc.tile_pool(name="psab", bufs=1, space="PSUM") as psab,
            tc.tile_pool(name="psm", bufs=2, space="PSUM") as psm,
            tc.tile_pool(name="dram", bufs=1, space="DRAM") as dpool,
        ):
            # ------------ constant tiles (DMAs emitted in priority order below) --
            wT_sb = cpool.tile([P, KC, C1], F32R)
            weff_sb = cpool.tile([P, KC, C1], F32)
            beff_sb = cpool.tile([1, C1], F32)
            ghi_sb = cpool.tile([CS, CSE], F32R)
            ghj_sb = cpool.tile([CS, CSE], F32R)
            ghjT_sb = cpool.tile([P, MC, CS], F32)
            eye_sb = cpool.tile([CS, CS], F32)
            eye1_sb = cpool.tile([1, 1], F32)

            def load_pm(t, nm):
                v = cpool.tile([P, MC], F32, tag=f"v_{nm}", name=f"v_{nm}")
                nc.sync.dma_start(v[:], t.ap())
                return v

            def load_consts2():
                nc.sync.dma_start(
                    weff_sb[:], weff.ap().rearrange("(ko p) o -> p ko o", p=P)
                )
                nc.sync.dma_start(beff_sb[:], beff.ap())
                nc.sync.dma_start(ghi_sb[:], g_hi.ap())
                nc.sync.dma_start(ghj_sb[:], g_hj.ap())
                nc.sync.dma_start(ghjT_sb[:], g_hjT.ap())
                nc.sync.dma_start(eye_sb[:], eye32.ap())
                nc.sync.dma_start(eye1_sb[:], eye1.ap())

            eps_col = cpool.tile([P, 1], F32)
            nc.vector.memset(eps_col[:], EPS)

            # ------------ per-sample state tiles ------------
            x_sb = [None] * SPC
            prod_sb = [None] * SPC
            xsel = [None] * SPC
            xsel_r = [None] * SPC
            idx_col = [None] * SPC
            idxf_t = [None] * SPC
            s1loc = cpool.tile([P, MC, SPC], F32)
            s2loc = cpool.tile([P, MC, SPC], F32)
            nc.vector.memset(s1loc[:], 0.0)
            nc.vector.memset(s2loc[:], 0.0)

            x_r = [None] * SPC
            for s in range(SPC):
                x_sb[s] = xpool.tile([P, KC, HWD], F32, tag=f"x{s}", name=f"x{s}")
                x_r[s] = xpool.tile([P, KC, HWD], F32R, tag=f"xr{s}", name=f"xr{s}")
                prod_sb[s] = prodpool.tile(
                    [P, MC, HWD], F32, tag=f"prod{s}", name=f"prod{s}"
                )

            def load_x(s):
                for k in range(KC):
                    nc.sync.dma_start(
                        x_sb[s][:, k, :],
                        xs.ap()[s].rearrange("(ko p) n -> p ko n", p=P)[:, k, :],
                    )

            def round_x(s):
                # rounded copy for the f32r conv matmuls (per k-chunk, on ACT)
                for k in range(KC):
                    nc.scalar.activation(
                        x_r[s][:, k, :], x_sb[s][:, k, :], AF.Copy
                    )

            def round_x_ops(s):
                def op(k):
                    return lambda: nc.scalar.activation(
                        x_r[s][:, k, :], x_sb[s][:, k, :], AF.Copy
                    )
                return [op(k) for k in range(KC)]

            def xsum_sample(s):
                # exact per-k hw-sums on DVE (start as soon as chunk k lands)
                xsum = cpool.tile([P, KC], F32, tag=f"xsum{s}", name=f"xsum{s}")
                for k in range(KC):
                    nc.vector.tensor_reduce(
                        xsum[:, k : k + 1], x_sb[s][:, k, :],
                        axis=mybir.AxisListType.X, op=ALU.add,
                    )
                return xsum

            # ================ conv (both samples, PE f32r) ================
            def conv_sample(s, act_fillers=None):
                for m in range(MC):
                    if act_fillers:
                        act_fillers.pop(0)()
                    yst = ypool.tile([P, HWD], F32, tag="yst")
                    for n in range(NNC):
                        pt = psc.tile([P, NF], F32, tag="cv")
                        for k in range(KC):
                            nc.tensor.matmul(
                                pt[:],
                                lhsT=wT_sb[:, k, m * P : (m + 1) * P],
                                rhs=x_r[s][:, k, n * NF : (n + 1) * NF],
                                start=(k == 0),
                                stop=(k == KC - 1),
                            )
                        nc.scalar.activation(
                            yst[:, n * NF : (n + 1) * NF], pt[:], AF.Identity,
                            bias=b2_v[:, m : m + 1], scale=s_v[:, m : m + 1],
                        )
                    nc.sync.dma_start(
                        outs[s].ap()[m * P : (m + 1) * P, :], yst[:]
                    )

            # ---- exact logits path (independent of conv results) ----
            def logits_topk(s, xsum):
                plt = psc.tile([P, NF], F32, tag="cv")
                pl = plt[0:1, :]
                for k in range(KC):
                    nc.tensor.matmul(
                        pl,
                        lhsT=xsum[:, k : k + 1],
                        rhs=weff_sb[:, k, :],
                        start=(k == 0),
                        stop=(k == KC - 1),
                    )
                logit = cpool.tile([1, C1], F32, tag=f"lg{s}", name=f"lg{s}")
                nc.vector.tensor_tensor(logit[:], pl, beff_sb[:], op=ALU.add)
                # top-32, descending
                idx_row = cpool.tile([1, CS], U32, tag=f"idxr{s}", name=f"idxr{s}")
                for r in range(4):
                    mx8 = spool.tile([1, 8], F32, tag=f"mx{s}")
                    nc.vector.max(out=mx8[:], in_=logit[:])
                    nc.vector.max_index(
                        out=idx_row[:, r * 8 : (r + 1) * 8], in_max=mx8[:],
                        in_values=logit[:],
                    )
                    if r < 3:
                        nc.vector.match_replace(
                            out=logit[:], in_to_replace=mx8[:], in_values=logit[:],
                            imm_value=-1e30,
                        )
                # row -> f32 (for the PE transpose, values <= 511)
                idxf = cpool.tile([1, CS], F32, tag=f"idxf{s}", name=f"idxf{s}")
                nc.vector.tensor_copy(idxf[:], idx_row[:])
                idxf_t[s] = idxf

            def idx_to_col(s):
                pidxt = psm.tile([P, NF], F32, tag="msc")
                nc.tensor.transpose(pidxt[0:CS, 0:1], idxf_t[s][:], eye1_sb[:])
                idx_col[s] = cpool.tile([CS, 1], U32, tag=f"idxc{s}", name=f"idxc{s}")
                nc.vector.tensor_copy(idx_col[s][:], pidxt[0:CS, 0:1])

            # ---- gather + Gram stats (early, gates the collective) ----
            def gather_stats(s):
                xsel[s] = cpool.tile([CS, HWD], F32, tag=f"xsel{s}", name=f"xsel{s}")
                nc.gpsimd.indirect_dma_start(
                    out=xsel[s][:],
                    out_offset=None,
                    in_=outs[s].ap()[0:C1, :],
                    in_offset=bass.IndirectOffsetOnAxis(ap=idx_col[s][:, :1], axis=0),
                )
                # rounded copy for the f32r hadamard matmuls
                xsel_r[s] = cpool.tile([CS, HWD], F32R, tag=f"xselr{s}", name=f"xselr{s}")
                nc.vector.tensor_copy(xsel_r[s][:], xsel[s][:])
                # transpose to [hw, 32] chunks; square on ACT
                ptr = psm.tile([P, NF], F32, tag="msc")
                for c in range(NHC):
                    nc.tensor.transpose(
                        ptr[:, c * CS : (c + 1) * CS],
                        xsel[s][:, c * P : (c + 1) * P], eye_sb[:],
                    )
                xT = cpool.tile([P, NHC, CS], F32, tag=f"xT{s}", name=f"xT{s}")
                nc.scalar.activation(xT[:], ptr[:, 0 : NHC * CS], AF.Copy)
                xsqT = cpool.tile([P, NHC, CS], F32, tag=f"xsqT{s}", name=f"xsqT{s}")
                nc.scalar.activation(xsqT[:], ptr[:, 0 : NHC * CS], AF.Square)
                pgt = psm.tile([P, NF], F32, tag="msc")
                pg = pgt[0:CS, 0:CS]
                pgt2 = psm.tile([P, NF], F32, tag="msc")
                pg2 = pgt2[0:CS, 0:CS]
                for c in range(NHC):
                    nc.tensor.matmul(
                        pg, lhsT=xT[:, c, :], rhs=xT[:, c, :],
                        start=(c == 0), stop=(c == NHC - 1),
                    )
                for c in range(NHC):
                    nc.tensor.matmul(
                        pg2, lhsT=xsqT[:, c, :], rhs=xsqT[:, c, :],
                        start=(c == 0), stop=(c == NHC - 1),
                    )
                gram = cpool.tile([CS, CS], F32, tag=f"gram{s}", name=f"gram{s}")
                nc.scalar.activation(gram[:], pg, AF.Copy)
                gram2 = cpool.tile([CS, CS], F32, tag=f"gram2{s}", name=f"gram2{s}")
                nc.scalar.activation(gram2[:], pg2, AF.Copy)
                # pair gather: s1[e] = Gram[hi[e], hj[e]] via Ghi^T @ Gram (.) GhjT
                for m, (mo, mw) in enumerate(EXP_M):
                    for gi, (gsb, sloc) in enumerate(((gram, s1loc), (gram2, s2loc))):
                        ppt = psm.tile([P, NF], F32, tag="msc")
                        pp = ppt[:, 0:CS]
                        nc.tensor.matmul(
                            pp[:mw, :],
                            lhsT=ghi_sb[:, mo : mo + mw].bitcast(F32),
                            rhs=gsb[:],
                            start=True, stop=True,
                        )
                        tmp = spool.tile([P, CS], F32, tag="pgt")
                        nc.vector.tensor_tensor(
                            tmp[:mw, :], pp[:mw, :], ghjT_sb[:mw, m, :], op=ALU.mult
                        )
                        nc.vector.tensor_reduce(
                            sloc[:mw, m, s : s + 1], tmp[:mw, :],
                            axis=mybir.AxisListType.X, op=ALU.add,
                        )

            # ---- hadamard A/B matmuls + product (overlaps collective) ----
            def hadamard(s):
                for n in range(NNC):
                    for m, (mo, mw) in enumerate(EXP_M):
                        pa = psab.tile([P, NF], F32, tag="pa")
                        pb = psab.tile([P, NF], F32, tag="pb")
                        nc.tensor.matmul(
                            pa[:mw, :],
                            lhsT=ghi_sb[:, mo : mo + mw],
                            rhs=xsel_r[s][:, n * NF : (n + 1) * NF],
                            start=True, stop=True,
                        )
                        nc.tensor.matmul(
                            pb[:mw, :],
                            lhsT=ghj_sb[:, mo : mo + mw],
                            rhs=xsel_r[s][:, n * NF : (n + 1) * NF],
                            start=True, stop=True,
                        )
                        a_sb = apool.tile([P, NF], F32, tag="ac")
                        nc.scalar.activation(a_sb[:mw, :], pa[:mw, :], AF.Copy)
                        nc.vector.tensor_tensor(
                            prod_sb[s][:mw, m, n * NF : (n + 1) * NF],
                            a_sb[:mw, :], pb[:mw, :], op=ALU.mult,
                        )

            # ================ emission order ================
            nc.sync.dma_start(
                wT_sb[:], fc_wT.ap().rearrange("(ko p) o -> p ko o", p=P)
            )
            load_x(0)
            s_v = load_pm(s_vec, "s")
            b2_v = load_pm(b2_vec, "b2")
            gpair = load_pm(gpair_t, "gp")
            bpair = load_pm(bpair_t, "bp")
            load_consts2()
            load_x(1)
            xs0 = xsum_sample(0)
            round_x(0)
            conv_sample(0, act_fillers=round_x_ops(1))
            logits_topk(0, xs0)
            idx_to_col(0)
            xs1 = xsum_sample(1)
            logits_topk(1, xs1)   # PE: pl1 only; topk1 on DVE overlaps conv1
            conv_sample(1)
            idx_to_col(1)
            gather_stats(0)
            gather_stats(1)

            # ---- pack stats + AllGather + alpha/beta ----
            stats = cpool.tile([P, 2 * MC], F32)
            nc.vector.tensor_reduce(
                stats[:, 0:MC], s1loc[:], axis=mybir.AxisListType.X, op=ALU.add
            )
            nc.vector.tensor_reduce(
                stats[:, MC : 2 * MC], s2loc[:], axis=mybir.AxisListType.X, op=ALU.add
            )
            cc_in = dpool.tile([P, 2 * MC], F32)
            cc_out = dpool.tile([P, 2 * MC], F32)
            nc.sync.dma_start(cc_in[:], stats[:])
            nc.gpsimd.collective_compute(
                "AllReduce",
                ALU.add,
                replica_groups=[list(range(NCORES))],
                ins=[cc_in.opt()],
                outs=[cc_out.opt()],
            )
            gath = cpool.tile([P, 2 * MC], F32)
            nc.sync.dma_start(gath[:], cc_out[:])

            hadamard(0)
            hadamard(1)

            gstats = gath
            meanc = cpool.tile([P, MC], F32)
            nc.vector.tensor_scalar_mul(meanc[:], gstats[:, 0:MC], 1.0 / NTOT)
            varc = cpool.tile([P, MC], F32)
            nc.vector.tensor_scalar_mul(varc[:], gstats[:, MC : 2 * MC], 1.0 / NTOT)
            msq = cpool.tile([P, MC], F32)
            nc.vector.tensor_mul(msq[:], meanc[:], meanc[:])
            nc.vector.tensor_sub(varc[:], varc[:], msq[:])
            nc.scalar.activation(varc[:], varc[:], AF.Sqrt, bias=eps_col[:, 0:1])
            rstd = cpool.tile([P, MC], F32)
            nc.vector.reciprocal(rstd[:], varc[:])
            alpha = cpool.tile([P, MC], F32)
            nc.vector.tensor_mul(alpha[:], rstd[:], gpair[:])
            beta2 = cpool.tile([P, MC], F32)
            nc.vector.tensor_mul(beta2[:], meanc[:], alpha[:])
            nc.vector.tensor_sub(beta2[:], bpair[:], beta2[:])

            # ---- final affine + prod writes (DVE/ACT split by m parity) ----
            for s in range(SPC):
                for m, (mo, mw) in enumerate(EXP_M):
                    pin = prod_sb[s][:mw, m, :]
                    ost = opool.tile([P, HWD], F32, tag="ost")
                    if m % 2 == 0:
                        nc.scalar.activation(
                            ost[:mw, :], pin, AF.Identity,
                            bias=beta2[:mw, m : m + 1],
                            scale=alpha[:mw, m : m + 1],
                        )
                    else:
                        nc.vector.tensor_scalar(
                            ost[:mw, :], pin,
                            alpha[:mw, m : m + 1],
                            beta2[:mw, m : m + 1],
                            op0=ALU.mult, op1=ALU.add,
                        )
                    nc.sync.dma_start(
                        outs[s].ap()[C1 + mo : C1 + mo + mw, :], ost[:mw, :]
                    )

    nc.compile()
    return nc


_NC_CACHE = {}


def _get_program():
    if "p" not in _NC_CACHE:
        _NC_CACHE["p"] = build_program()
    return _NC_CACHE["p"]


def _make_consts(fc_w, fc_b, bn_gamma, bn_beta, bn_mean, bn_var, eva_w, eva_b):
    s64 = bn_gamma.astype(np.float64) / np.sqrt(bn_var.astype(np.float64) + EPS)
    b264 = (fc_b.astype(np.float64) - bn_mean.astype(np.float64)) * s64 + bn_beta.astype(
        np.float64
    )
    # logits = xbar @ fc_w.T * s + b2) @ eva_w.T + eva_b ; fold mean 1/HWD
    weff = ((fc_w.T.astype(np.float64) * s64[None, :]) @ eva_w.T.astype(np.float64)) / HWD
    beff = b264 @ eva_w.T.astype(np.float64) + eva_b.astype(np.float64)

    ghi = np.zeros((CS, CSE), np.float32)
    ghj = np.zeros((CS, CSE), np.float32)
    ghi[HI, np.arange(CSE)] = 1.0
    ghj[HJ, np.arange(CSE)] = 1.0
    ghjT = np.zeros((P, MC, CS), np.float32)
    for m, (mo, mw) in enumerate(EXP_M):
        for p in range(mw):
            ghjT[p, m, HJ[mo + p]] = 1.0

    def pm(vec):
        out = np.zeros((P, MC), np.float32)
        out[:, :] = vec.astype(np.float32).reshape(MC, P).T
        return out

    gp = np.zeros(C1, np.float64)
    bp = np.zeros(C1, np.float64)
    gp[:CSE] = bn_gamma[HI].astype(np.float64) * bn_gamma[HJ].astype(np.float64)
    bp[:CSE] = bn_beta[HI].astype(np.float64) * bn_beta[HJ].astype(np.float64)
    gpair = np.zeros((P, MC), np.float32)
    bpair = np.zeros((P, MC), np.float32)
    for m, (mo, mw) in enumerate(EXP_M):
        gpair[:mw, m] = gp[mo : mo + mw].astype(np.float32)
        bpair[:mw, m] = bp[mo : mo + mw].astype(np.float32)

    return dict(
        fc_wT=np.ascontiguousarray(fc_w.T.astype(np.float32)),
        weff=np.ascontiguousarray(weff.astype(np.float32)),
        beff=beff.astype(np.float32).reshape(1, C1),
        s_vec=pm(np.asarray(s64)),
        b2_vec=pm(np.asarray(b264)),
        gpair_t=gpair,
        bpair_t=bpair,
        g_hi=ghi,
        g_hj=ghj,
        g_hjT=ghjT,
        eye32=np.eye(CS, dtype=np.float32),
        eye1=np.ones((1, 1), np.float32),
    )


def make_in_maps(inputs):
    x = np.asarray(inputs["x"], np.float32).reshape(B, C1, HWD)
    shared = _make_consts(
        np.asarray(inputs["fc_w"], np.float32), np.asarray(inputs["fc_b"], np.float32),
        np.asarray(inputs["bn_gamma"], np.float32),
        np.asarray(inputs["bn_beta"], np.float32),
        np.asarray(inputs["bn_mean"], np.float32),
        np.asarray(inputs["bn_var"], np.float32),
        np.asarray(inputs["eva_w"], np.float32), np.asarray(inputs["eva_b"], np.float32),
    )
    return [
        dict(shared, xs=np.ascontiguousarray(x[i * SPC : (i + 1) * SPC]))
        for i in range(NCORES)
    ]


def kernel(x, fc_w, fc_b, bn_gamma, bn_beta, bn_mean, bn_var, eva_w, eva_b):
    in_maps = make_in_maps(dict(
        x=x, fc_w=fc_w, fc_b=fc_b, bn_gamma=bn_gamma, bn_beta=bn_beta,
        bn_mean=bn_mean, bn_var=bn_var, eva_w=eva_w, eva_b=eva_b,
    ))
    nc = _get_program()
    res = run_bass_kernel_spmd(nc, in_maps, list(range(NCORES))).results
    out = np.empty((B, C1 + CSE, HWD), np.float32)
    for i in range(NCORES):
        for s in range(SPC):
            out[i * SPC + s] = res[i][f"out{s}"]
    return out.reshape(B, C1 + CSE, 32, 32)
